# revision 11
# baseline (speedup 1.0000x reference)
"""Causal attention (B=4, S=2048, D=1024, single 1024-dim head) on 8 TRN2 cores.

Sharding: data-parallel over batch (4) x 2-way causal-balanced query split
(zigzag 256-row query blocks: core A gets global blocks {0,3,4,7}, core B
{1,2,5,6}).  Every core runs the same SPMD program over 4 query slots with
k-chunk counts {4,8,12,16}; causality differences between the cores are
expressed purely through per-core input data (gathered q columns + mask
tiles), never through the instruction stream.

Device algorithm (transposed layouts throughout so every matmul contracts
over the partition dim with naturally-DMA-able operands):
  qT = (Wq xT_gathered)          [dout, 1024]  via lhsT=WqT chunks, rhs=xqT
  kT = (Wk xT)                   [dout, 2048]
  per q-slot (256 cols), per k-chunk (128 rows):
      sT   = kT_chunk^T qT_slot  [128k, 256q]   (PSUM, 8 dout-chunk matmuls)
      expT = exp(sT/32)          (ACT, PSUM->SBUF; scores are O(+-8) so no
                                  max-subtraction is needed)
      mask-multiply (DVE) for the last 4 chunks of the slot (host tiles)
      dacc += expT               (DVE; softmax denominator partial sums)
      ctxT[d] += xn_chunk[:,d]^T expT   (PSUM accumulate: ctx = attn @ x,
                                  using attn@x@WvT == attn@(x WvT) assoc.)
  denom: ones^T dacc (matmul) -> reciprocal -> broadcast matmul -> DVE mul
  outT = WvT^T ctxN              [dout, 1024]
All matmuls run as float32r (fp32 storage, ~tf32 precision, 4x the fp32
matmul rate).
"""

import os
import sys

sys.path.insert(0, "/opt/trn_rl_repo")

import numpy as np

B, S, DIN, DOUT = 4, 2048, 1024, 1024
P = 128
NQ = 1024  # q rows per core
ND = DIN // P
NO = DOUT // P
NCORES = 8
G = [[0, 3, 4, 7], [1, 2, 5, 6]]  # global 256-row q-block per (core-half, slot)
L = [4, 8, 12, 16]  # k-chunks processed per slot (uniform across cores)

_NC_CACHE = {}


def _build_nc():
    import concourse.mybir as mybir
    import concourse.tile as tile
    from concourse import bacc
    from contextlib import ExitStack

    f32 = mybir.dt.float32
    f32r = mybir.dt.float32r
    EXP = mybir.ActivationFunctionType.Exp

    nc = bacc.Bacc("TRN2", target_bir_lowering=False, debug=False,
                   num_devices=NCORES)

    xqT_d = nc.dram_tensor("xqT", [DIN, NQ], f32, kind="ExternalInput").ap()
    xT_d = nc.dram_tensor("xT", [DIN, S], f32, kind="ExternalInput").ap()
    xn_d = nc.dram_tensor("xn", [S, DIN], f32, kind="ExternalInput").ap()
    wqT_d = nc.dram_tensor("wqT", [DIN, DOUT], f32, kind="ExternalInput").ap()
    wkT_d = nc.dram_tensor("wkT", [DIN, DOUT], f32, kind="ExternalInput").ap()
    wvT_d = nc.dram_tensor("wvT", [DIN, DOUT], f32, kind="ExternalInput").ap()
    masks_d = nc.dram_tensor("masks", [P, 16 * 256], f32, kind="ExternalInput").ap()
    ones_d = nc.dram_tensor("ones", [P, 160], f32, kind="ExternalInput").ap()
    outT_d = nc.dram_tensor("outT", [DOUT, NQ], f32, kind="ExternalOutput").ap()

    with tile.TileContext(nc) as tc:
        with ExitStack() as es:
            qT_pool = es.enter_context(tc.tile_pool(name="qTp", bufs=1))
            kT_pool = es.enter_context(tc.tile_pool(name="kTp", bufs=1))
            ctx_pool = es.enter_context(tc.tile_pool(name="ctxp", bufs=1))
            cst_pool = es.enter_context(tc.tile_pool(name="cst", bufs=1))

            qT = [qT_pool.tile([P, NQ], f32r, name=f"qT{o}", tag=f"qT{o}")
                  for o in range(NO)]
            kT = [kT_pool.tile([P, S], f32r, name=f"kT{o}", tag=f"kT{o}")
                  for o in range(NO)]
            onesT = cst_pool.tile([P, 160], f32r, name="onesT", tag="onesT")
            nc.sync.dma_start(onesT[:], ones_d[:].bitcast(f32r))
            ones_col = onesT[:, 0:1]      # [128, 1]
            ones_row = onesT[0:1, 32:160]  # [1, 128]

            # ---------------- phase 1: q/k projections ----------------
            with tc.tile_pool(name="xs", bufs=8) as x_pool, \
                 tc.tile_pool(name="ws", bufs=8) as w_pool, \
                 tc.tile_pool(name="pps", bufs=4, space="PSUM") as proj_ps:
                # 1a: qT = Wq @ x^T (gathered q cols)
                for d in range(ND):
                    xq = x_pool.tile([P, NQ], f32r, name=f"xq{d}", tag="xs")
                    nc.sync.dma_start(
                        xq[:], xqT_d[d * P:(d + 1) * P, :].bitcast(f32r))
                    if d == 0:
                        xqs = []
                    xqs.append(xq)
                wqs = []
                for d in range(ND):
                    wq = w_pool.tile([P, DOUT], f32r, name=f"wq{d}", tag="ws")
                    nc.sync.dma_start(
                        wq[:], wqT_d[d * P:(d + 1) * P, :].bitcast(f32r))
                    wqs.append(wq)
                for o in range(NO):
                    for h in range(2):
                        po = proj_ps.tile([P, 512], f32, name="poq", tag="po")
                        for d in range(ND):
                            nc.tensor.matmul(
                                po[:],
                                wqs[d][:, o * P:(o + 1) * P],
                                xqs[d][:, h * 512:(h + 1) * 512],
                                start=(d == 0), stop=(d == ND - 1))
                        nc.vector.tensor_copy(qT[o][:, h * 512:(h + 1) * 512], po[:])

                # 1b: kT = Wk @ x^T, all 2048 keys in two column halves
                wks = []
                for d in range(ND):
                    wk = w_pool.tile([P, DOUT], f32r, name=f"wk{d}", tag="ws")
                    nc.sync.dma_start(
                        wk[:], wkT_d[d * P:(d + 1) * P, :].bitcast(f32r))
                    wks.append(wk)
                for half in range(2):
                    xhs = []
                    for d in range(ND):
                        xh = x_pool.tile([P, 1024], f32r, name=f"xh{half}_{d}",
                                         tag="xs")
                        nc.sync.dma_start(
                            xh[:],
                            xT_d[d * P:(d + 1) * P,
                                 half * 1024:(half + 1) * 1024].bitcast(f32r))
                        xhs.append(xh)
                    for o in range(NO):
                        for kp in range(2):
                            po = proj_ps.tile([P, 512], f32, name="pok", tag="po")
                            for d in range(ND):
                                nc.tensor.matmul(
                                    po[:],
                                    wks[d][:, o * P:(o + 1) * P],
                                    xhs[d][:, kp * 512:(kp + 1) * 512],
                                    start=(d == 0), stop=(d == ND - 1))
                            col = half * 1024 + kp * 512
                            nc.vector.tensor_copy(kT[o][:, col:col + 512], po[:])

            # ---------------- phase 2: attention ----------------
            with tc.tile_pool(name="msk", bufs=1) as msk_pool, \
                 tc.tile_pool(name="xnp", bufs=4) as xn_pool, \
                 tc.tile_pool(name="exq", bufs=4) as exp_pool, \
                 tc.tile_pool(name="dac", bufs=2) as dacc_pool, \
                 tc.tile_pool(name="sml", bufs=2) as small_pool, \
                 tc.tile_pool(name="sps", bufs=2, space="PSUM") as sT_ps, \
                 tc.tile_pool(name="cps", bufs=4, space="PSUM") as ctx_ps, \
                 tc.tile_pool(name="dps", bufs=1, space="PSUM") as dn_ps:
                maskT = msk_pool.tile([P, 16 * 256], f32r, name="maskT",
                                      tag="maskT")
                nc.sync.dma_start(maskT[:], masks_d[:].bitcast(f32r))
                ctxN = [ctx_pool.tile([P, NQ], f32r, name=f"ctx{d}",
                                      tag=f"ctx{d}") for d in range(ND)]
                for s in range(4):
                    q0 = s * 256
                    cps = [ctx_ps.tile([P, 512], f32, name=f"cps{s}_{i}",
                                       tag="cps") for i in range(4)]
                    dacc = dacc_pool.tile([P, 256], f32r, name="dacc", tag="dacc")
                    for c in range(L[s]):
                        st = sT_ps.tile([P, 256], f32, name="st", tag="st")
                        for o in range(NO):
                            nc.tensor.matmul(
                                st[:],
                                kT[o][:, c * P:(c + 1) * P],
                                qT[o][:, q0:q0 + 256],
                                start=(o == 0), stop=(o == NO - 1))
                        et = exp_pool.tile([P, 256], f32r, name="et", tag="et")
                        nc.scalar.activation(et[:], st[:], EXP, scale=1.0 / 32.0)
                        if c >= L[s] - 4:
                            m = 4 * s + (c - (L[s] - 4))
                            et2 = exp_pool.tile([P, 256], f32r, name="et2",
                                                tag="et2")
                            nc.vector.tensor_mul(
                                et2[:], et[:], maskT[:, m * 256:(m + 1) * 256])
                            et = et2
                        if c == 0:
                            nc.vector.tensor_copy(dacc[:], et[:])
                        else:
                            nc.vector.tensor_add(dacc[:], dacc[:], et[:])
                        xnt = xn_pool.tile([P, DIN], f32r, name="xnt", tag="xnt")
                        nc.sync.dma_start(
                            xnt[:], xn_d[c * P:(c + 1) * P, :].bitcast(f32r))
                        # Two accumulators share each PSUM bank; start=True
                        # clears has_written for the WHOLE bank, so only the
                        # first (even-d) group may use it.  The odd-d group
                        # initializes via start=False (bits are clear after
                        # the even start, so its first matmul overwrites),
                        # which requires the c==0 matmuls to execute in d
                        # order -> pin them with tile_critical.
                        if c == 0:
                            with tc.tile_critical():
                                for d in range(ND):
                                    acc = cps[d // 2][:, (d % 2) * 256:
                                                      (d % 2) * 256 + 256]
                                    nc.tensor.matmul(
                                        acc, xnt[:, d * P:(d + 1) * P], et[:],
                                        start=(d % 2 == 0),
                                        stop=(L[s] == 1))
                        else:
                            for d in range(ND):
                                acc = cps[d // 2][:, (d % 2) * 256:
                                                  (d % 2) * 256 + 256]
                                nc.tensor.matmul(
                                    acc, xnt[:, d * P:(d + 1) * P], et[:],
                                    start=False, stop=(c == L[s] - 1))
                    # softmax denominator -> reciprocal -> partition-broadcast
                    dsum = dn_ps.tile([1, 256], f32, name="dsum", tag="dsum")
                    nc.tensor.matmul(dsum[:], ones_col, dacc[:],
                                     start=True, stop=True)
                    rec = small_pool.tile([1, 256], f32r, name="rec", tag="rec")
                    with nc.allow_low_precision(
                            reason="f32r reciprocal feeds f32r bcast matmul"):
                        nc.vector.reciprocal(rec[:], dsum[:])
                    bc = dn_ps.tile([P, 256], f32, name="bc", tag="bc")
                    nc.tensor.matmul(bc[:], ones_row, rec[:],
                                     start=True, stop=True)
                    bcs = small_pool.tile([P, 256], f32, name="bcs", tag="bcs")
                    nc.vector.tensor_copy(bcs[:], bc[:])
                    for d in range(ND):
                        src = cps[d // 2][:, (d % 2) * 256:(d % 2) * 256 + 256]
                        nc.vector.tensor_mul(ctxN[d][:, q0:q0 + 256], src, bcs[:])

            # ---------------- phase 3: out^T = Wv ctx^T ----------------
            with tc.tile_pool(name="wvp", bufs=8) as wv_pool, \
                 tc.tile_pool(name="obp", bufs=4) as out_pool, \
                 tc.tile_pool(name="ops", bufs=4, space="PSUM") as out_ps:
                wvs = []
                for d in range(ND):
                    wv = wv_pool.tile([P, DOUT], f32r, name=f"wv{d}", tag="wv")
                    nc.sync.dma_start(
                        wv[:], wvT_d[d * P:(d + 1) * P, :].bitcast(f32r))
                    wvs.append(wv)
                for o in range(NO):
                    for qp in range(2):
                        po = out_ps.tile([P, 512], f32, name="poo", tag="poo")
                        for d in range(ND):
                            nc.tensor.matmul(
                                po[:],
                                wvs[d][:, o * P:(o + 1) * P],
                                ctxN[d][:, qp * 512:(qp + 1) * 512],
                                start=(d == 0), stop=(d == ND - 1))
                        ob = out_pool.tile([P, 512], f32, name="ob", tag="ob")
                        nc.vector.tensor_copy(ob[:], po[:])
                        nc.sync.dma_start(
                            outT_d[o * P:(o + 1) * P, qp * 512:(qp + 1) * 512],
                            ob[:])

    nc.compile()
    return nc


def _get_nc():
    if "nc" not in _NC_CACHE:
        _NC_CACHE["nc"] = _build_nc()
    return _NC_CACHE["nc"]


def _make_masks(h):
    """[128, 16*256] mask tile row: 1.0 where key 128c+p <= query 256g+j."""
    mk = np.zeros((P, 16 * 256), dtype=np.float32)
    p = np.arange(P)[:, None]
    j = np.arange(256)[None, :]
    for s in range(4):
        g = G[h][s]
        for m in range(4):
            c = L[s] - 4 + m
            mk[:, (4 * s + m) * 256:(4 * s + m + 1) * 256] = (
                (128 * c + p) <= (256 * g + j)).astype(np.float32)
    return mk


def kernel(x, W_q, W_k, W_v):
    from concourse.bass_utils import run_bass_kernel_spmd

    x = np.ascontiguousarray(np.asarray(x, dtype=np.float32))
    wqT = np.ascontiguousarray(np.asarray(W_q, dtype=np.float32).T)
    wkT = np.ascontiguousarray(np.asarray(W_k, dtype=np.float32).T)
    wvT = np.ascontiguousarray(np.asarray(W_v, dtype=np.float32).T)

    ones = np.zeros((P, 160), dtype=np.float32)
    ones[:, 0] = 1.0
    ones[0, 32:160] = 1.0
    masks_h = [_make_masks(0), _make_masks(1)]

    in_maps = []
    for b in range(B):
        xTb = np.ascontiguousarray(x[b].T)
        for h in range(2):
            qcols = np.concatenate(
                [np.arange(g * 256, (g + 1) * 256) for g in G[h]])
            in_maps.append(dict(
                xqT=np.ascontiguousarray(xTb[:, qcols]),
                xT=xTb,
                xn=x[b],
                wqT=wqT, wkT=wkT, wvT=wvT,
                masks=masks_h[h],
                ones=ones,
            ))

    nc = _get_nc()
    res = run_bass_kernel_spmd(nc, in_maps, core_ids=list(range(NCORES)),
                               trace=bool(os.environ.get("KERNEL_TRACE")))
    if os.environ.get("KERNEL_TRACE"):
        _NC_CACHE["last_results"] = res

    out = np.empty((B, S, DOUT), dtype=np.float32)
    for b in range(B):
        for h in range(2):
            oT = res.results[b * 2 + h]["outT"]
            for s2, g in enumerate(G[h]):
                out[b, g * 256:(g + 1) * 256, :] = \
                    oT[:, s2 * 256:(s2 + 1) * 256].T
    return out


# revision 15
# speedup vs baseline: 1.1221x; 1.1221x over previous
"""Causal attention (B=4, S=2048, D=1024, single 1024-dim head) on 8 TRN2 cores.

Sharding: data-parallel over batch (4) x 2-way causal-balanced query split
(zigzag 256-row query blocks: core A gets global blocks {0,3,4,7}, core B
{1,2,5,6}).  Every core runs the same SPMD program over 4 query slots with
k-chunk counts {4,8,12,16}; causality differences between the cores are
expressed purely through per-core input data (gathered q columns + mask
tiles), never through the instruction stream.

Device algorithm (transposed layouts throughout so every matmul contracts
over the partition dim with naturally-DMA-able operands):
  kT = (Wk xT)                   [dout, 2048]
  qT = (Wq xT_gathered)          [dout, 1024]
  per q-slot (256 cols), per k-chunk (128 rows):
      sT   = kT_chunk^T qT_slot  [128k, 256q]   (PSUM, 8 dout-chunk matmuls)
      expT = exp(sT/32)          (ACT, PSUM->SBUF; scores are O(+-8) so no
                                  max-subtraction is needed)
      mask-multiply (DVE) for the last 4 chunks of the slot (host tiles)
      dacc += expT               (DVE; softmax denominator partial sums)
      ctxU[d] += xn_chunk[:,d]^T expT   (PSUM accumulate: ctx = attn @ x,
                                  using attn@x@WvT == attn@(x WvT) assoc.)
  denom per slot: ones^T dacc (matmul) -> reciprocal -> broadcast matmul
                  -> recip row stored in bcsAll (normalization is linear, so
                  it is deferred to the phase-3 PSUM evacuation multiply)
  outT = (WvT^T ctxU) * bcsAll   [dout, 1024]
All matmuls run as float32r (fp32 storage, ~tf32 precision, 4x the fp32
matmul rate).
"""

import os
import sys

sys.path.insert(0, "/opt/trn_rl_repo")

import numpy as np

B, S, DIN, DOUT = 4, 2048, 1024, 1024
P = 128
NQ = 1024  # q rows per core
ND = DIN // P
NO = DOUT // P
NCORES = 8
G = [[0, 3, 4, 7], [1, 2, 5, 6]]  # global 256-row q-block per (core-half, slot)
L = [4, 8, 12, 16]  # k-chunks processed per slot (uniform across cores)

_NC_CACHE = {}


def _build_nc():
    import concourse.mybir as mybir
    import concourse.tile as tile
    from concourse import bacc
    from contextlib import ExitStack

    f32 = mybir.dt.float32
    f32r = mybir.dt.float32r
    EXP = mybir.ActivationFunctionType.Exp

    nc = bacc.Bacc("TRN2", target_bir_lowering=False, debug=False,
                   num_devices=NCORES)

    xqT_d = nc.dram_tensor("xqT", [DIN, NQ], f32, kind="ExternalInput").ap()
    xT_d = nc.dram_tensor("xT", [DIN, S], f32, kind="ExternalInput").ap()
    xn_d = nc.dram_tensor("xn", [S, DIN], f32, kind="ExternalInput").ap()
    wqT_d = nc.dram_tensor("wqT", [DIN, DOUT], f32, kind="ExternalInput").ap()
    wkT_d = nc.dram_tensor("wkT", [DIN, DOUT], f32, kind="ExternalInput").ap()
    wvT_d = nc.dram_tensor("wvT", [DIN, DOUT], f32, kind="ExternalInput").ap()
    masks_d = nc.dram_tensor("masks", [P, 16 * 256], f32, kind="ExternalInput").ap()
    ones_d = nc.dram_tensor("ones", [P, 160], f32, kind="ExternalInput").ap()
    outT_d = nc.dram_tensor("outT", [DOUT, NQ], f32, kind="ExternalOutput").ap()

    with tile.TileContext(nc) as tc:
        with ExitStack() as es:
            qT_pool = es.enter_context(tc.tile_pool(name="qTp", bufs=1))
            kT_pool = es.enter_context(tc.tile_pool(name="kTp", bufs=1))
            ctx_pool = es.enter_context(tc.tile_pool(name="ctxp", bufs=1))
            cst_pool = es.enter_context(tc.tile_pool(name="cst", bufs=1))

            qT = [qT_pool.tile([P, NQ], f32r, name=f"qT{o}", tag=f"qT{o}")
                  for o in range(NO)]
            kT = [kT_pool.tile([P, S], f32r, name=f"kT{o}", tag=f"kT{o}")
                  for o in range(NO)]
            onesT = cst_pool.tile([P, 160], f32r, name="onesT", tag="onesT")
            nc.sync.dma_start(onesT[:], ones_d[:].bitcast(f32r))
            ones_col = onesT[:, 0:1]      # [128, 1]
            ones_row = onesT[0:1, 32:160]  # [1, 128]
            # per-slot softmax 1/denominator rows, partition-broadcast
            bcsAll = cst_pool.tile([P, NQ], f32, name="bcsAll", tag="bcsAll")

            # ---------------- phase 1: k/q projections ----------------
            with tc.tile_pool(name="xs", bufs=10) as x_pool, \
                 tc.tile_pool(name="ws", bufs=8) as w_pool, \
                 tc.tile_pool(name="pps", bufs=5, space="PSUM") as proj_ps:
                # PE warmup during the initial DMA head: harmless matmuls on
                # the (tiny, loaded-first) ones tile keep the HAM clock gate
                # from idling while the first x/W tiles stream in.
                wu = proj_ps.tile([P, 128], f32, name="wu", tag="wu", bufs=1)
                for r in range(32):
                    nc.tensor.matmul(wu[:], onesT[:, 0:128], onesT[:, 0:128],
                                     start=True, stop=True,
                                     skip_group_check=True)

                # 1a: kT = Wk @ x^T, all 2048 keys in two column halves.
                # d-outer loop with 4 concurrent PSUM chains -> each weight
                # slice (lhsT) is loaded once per 4 matmuls.
                wks = []
                for d in range(ND):
                    wk = w_pool.tile([P, DOUT], f32r, name=f"wk{d}", tag="ws")
                    nc.sync.dma_start(
                        wk[:], wkT_d[d * P:(d + 1) * P, :].bitcast(f32r))
                    wks.append(wk)
                for half in range(2):
                    xhs = []
                    for d in range(ND):
                        xh = x_pool.tile([P, 1024], f32r, name=f"xh{half}_{d}",
                                         tag="xs")
                        nc.sync.dma_start(
                            xh[:],
                            xT_d[d * P:(d + 1) * P,
                                 half * 1024:(half + 1) * 1024].bitcast(f32r))
                        xhs.append(xh)
                    for o in range(NO):
                        pos = [proj_ps.tile([P, 512], f32, name=f"pok{kp}",
                                            tag="po") for kp in range(2)]
                        for d in range(ND):
                            for kp in range(2):
                                nc.tensor.matmul(
                                    pos[kp][:],
                                    wks[d][:, o * P:(o + 1) * P],
                                    xhs[d][:, kp * 512:(kp + 1) * 512],
                                    start=(d == 0), stop=(d == ND - 1))
                        for kp in range(2):
                            col = half * 1024 + kp * 512
                            nc.vector.tensor_copy(kT[o][:, col:col + 512],
                                                  pos[kp][:])

                # 1b: qT = Wq @ x^T (gathered q cols)
                xqs = []
                for d in range(ND):
                    xq = x_pool.tile([P, NQ], f32r, name=f"xq{d}", tag="xs")
                    nc.sync.dma_start(
                        xq[:], xqT_d[d * P:(d + 1) * P, :].bitcast(f32r))
                    xqs.append(xq)
                wqs = []
                for d in range(ND):
                    wq = w_pool.tile([P, DOUT], f32r, name=f"wq{d}", tag="ws")
                    nc.sync.dma_start(
                        wq[:], wqT_d[d * P:(d + 1) * P, :].bitcast(f32r))
                    wqs.append(wq)
                for o in range(NO):
                    pos = [proj_ps.tile([P, 512], f32, name=f"poq{h}",
                                        tag="po") for h in range(2)]
                    for d in range(ND):
                        for h in range(2):
                            nc.tensor.matmul(
                                pos[h][:],
                                wqs[d][:, o * P:(o + 1) * P],
                                xqs[d][:, h * 512:(h + 1) * 512],
                                start=(d == 0), stop=(d == ND - 1))
                    for h in range(2):
                        nc.vector.tensor_copy(qT[o][:, h * 512:(h + 1) * 512],
                                              pos[h][:])

            # ---------------- phase 2: attention ----------------
            with tc.tile_pool(name="msk", bufs=1) as msk_pool, \
                 tc.tile_pool(name="xnp", bufs=6) as xn_pool, \
                 tc.tile_pool(name="exq", bufs=4) as exp_pool, \
                 tc.tile_pool(name="dac", bufs=2) as dacc_pool, \
                 tc.tile_pool(name="sml", bufs=2) as small_pool, \
                 tc.tile_pool(name="sps", bufs=2, space="PSUM") as sT_ps, \
                 tc.tile_pool(name="cps", bufs=4, space="PSUM") as ctx_ps, \
                 tc.tile_pool(name="dps", bufs=1, space="PSUM") as dn_ps:
                maskT = msk_pool.tile([P, 16 * 256], f32r, name="maskT",
                                      tag="maskT")
                nc.sync.dma_start(maskT[:], masks_d[:].bitcast(f32r))
                ctxN = [ctx_pool.tile([P, NQ], f32r, name=f"ctx{d}",
                                      tag=f"ctx{d}") for d in range(ND)]
                for s in range(4):
                    q0 = s * 256
                    cps = [ctx_ps.tile([P, 512], f32, name=f"cps{s}_{i}",
                                       tag="cps") for i in range(4)]
                    dacc = dacc_pool.tile([P, 256], f32r, name="dacc", tag="dacc")
                    for c in range(L[s]):
                        st = sT_ps.tile([P, 256], f32, name="st", tag="st")
                        for o in range(NO):
                            nc.tensor.matmul(
                                st[:],
                                kT[o][:, c * P:(c + 1) * P],
                                qT[o][:, q0:q0 + 256],
                                start=(o == 0), stop=(o == NO - 1))
                        et = exp_pool.tile([P, 256], f32r, name="et", tag="et")
                        nc.scalar.activation(et[:], st[:], EXP, scale=1.0 / 32.0)
                        if c >= L[s] - 4:
                            m = 4 * s + (c - (L[s] - 4))
                            et2 = exp_pool.tile([P, 256], f32r, name="et2",
                                                tag="et2")
                            nc.vector.tensor_mul(
                                et2[:], et[:], maskT[:, m * 256:(m + 1) * 256])
                            et = et2
                        if c == 0:
                            nc.vector.tensor_copy(dacc[:], et[:])
                        else:
                            nc.vector.tensor_add(dacc[:], dacc[:], et[:])
                        xnt = xn_pool.tile([P, DIN], f32r, name="xnt", tag="xnt")
                        nc.sync.dma_start(
                            xnt[:], xn_d[c * P:(c + 1) * P, :].bitcast(f32r))
                        # Two accumulators share each PSUM bank; start=True
                        # clears has_written for the WHOLE bank, so only the
                        # first (even-d) group may use it.  The odd-d group
                        # initializes via start=False (bits are clear after
                        # the even start, so its first matmul overwrites),
                        # which requires the c==0 matmuls to execute in d
                        # order -> pin them with tile_critical.
                        if c == 0:
                            with tc.tile_critical():
                                for d in range(ND):
                                    acc = cps[d // 2][:, (d % 2) * 256:
                                                      (d % 2) * 256 + 256]
                                    nc.tensor.matmul(
                                        acc, xnt[:, d * P:(d + 1) * P], et[:],
                                        start=(d % 2 == 0),
                                        stop=(L[s] == 1))
                        else:
                            for d in range(ND):
                                acc = cps[d // 2][:, (d % 2) * 256:
                                                  (d % 2) * 256 + 256]
                                nc.tensor.matmul(
                                    acc, xnt[:, d * P:(d + 1) * P], et[:],
                                    start=False, stop=(c == L[s] - 1))
                    # Evacuate the ctx accumulators with plain copies (frees
                    # the PSUM banks for the next slot without waiting on the
                    # denominator chain).
                    for d in range(ND):
                        src = cps[d // 2][:, (d % 2) * 256:(d % 2) * 256 + 256]
                        nc.vector.tensor_copy(ctxN[d][:, q0:q0 + 256], src)
                    # softmax denominator -> reciprocal -> partition-broadcast
                    dsum = dn_ps.tile([1, 256], f32, name="dsum", tag="dsum")
                    nc.tensor.matmul(dsum[:], ones_col, dacc[:],
                                     start=True, stop=True)
                    rec = small_pool.tile([1, 256], f32r, name="rec", tag="rec")
                    with nc.allow_low_precision(
                            reason="f32r reciprocal feeds f32r bcast matmul"):
                        nc.vector.reciprocal(rec[:], dsum[:])
                    bc = dn_ps.tile([P, 256], f32, name="bc", tag="bc")
                    nc.tensor.matmul(bc[:], ones_row, rec[:],
                                     start=True, stop=True)
                    nc.vector.tensor_copy(bcsAll[:, q0:q0 + 256], bc[:])

            # ------- phase 3: out^T = (Wv ctx^T) * (1/denominator) -------
            with tc.tile_pool(name="wvp", bufs=8) as wv_pool, \
                 tc.tile_pool(name="obp", bufs=4) as out_pool, \
                 tc.tile_pool(name="ops", bufs=5, space="PSUM") as out_ps:
                wvs = []
                for d in range(ND):
                    wv = wv_pool.tile([P, DOUT], f32r, name=f"wv{d}", tag="wv")
                    nc.sync.dma_start(
                        wv[:], wvT_d[d * P:(d + 1) * P, :].bitcast(f32r))
                    wvs.append(wv)
                for o in range(NO):
                    pos = [out_ps.tile([P, 512], f32, name=f"poo{qp}",
                                       tag="poo") for qp in range(2)]
                    for d in range(ND):
                        for qp in range(2):
                            nc.tensor.matmul(
                                pos[qp][:],
                                wvs[d][:, o * P:(o + 1) * P],
                                ctxN[d][:, qp * 512:(qp + 1) * 512],
                                start=(d == 0), stop=(d == ND - 1))
                    for qp in range(2):
                        ob = out_pool.tile([P, 512], f32, name="ob", tag="ob")
                        nc.vector.tensor_mul(
                            ob[:], pos[qp][:],
                            bcsAll[:, qp * 512:(qp + 1) * 512])
                        nc.sync.dma_start(
                            outT_d[o * P:(o + 1) * P, qp * 512:(qp + 1) * 512],
                            ob[:])

    nc.compile()
    return nc


def _get_nc():
    if "nc" not in _NC_CACHE:
        _NC_CACHE["nc"] = _build_nc()
    return _NC_CACHE["nc"]


def _make_masks(h):
    """[128, 16*256] mask tile row: 1.0 where key 128c+p <= query 256g+j."""
    mk = np.zeros((P, 16 * 256), dtype=np.float32)
    p = np.arange(P)[:, None]
    j = np.arange(256)[None, :]
    for s in range(4):
        g = G[h][s]
        for m in range(4):
            c = L[s] - 4 + m
            mk[:, (4 * s + m) * 256:(4 * s + m + 1) * 256] = (
                (128 * c + p) <= (256 * g + j)).astype(np.float32)
    return mk


def kernel(x, W_q, W_k, W_v):
    from concourse.bass_utils import run_bass_kernel_spmd

    x = np.ascontiguousarray(np.asarray(x, dtype=np.float32))
    wqT = np.ascontiguousarray(np.asarray(W_q, dtype=np.float32).T)
    wkT = np.ascontiguousarray(np.asarray(W_k, dtype=np.float32).T)
    wvT = np.ascontiguousarray(np.asarray(W_v, dtype=np.float32).T)

    ones = np.zeros((P, 160), dtype=np.float32)
    ones[:, 0] = 1.0
    ones[0, 32:160] = 1.0
    masks_h = [_make_masks(0), _make_masks(1)]

    in_maps = []
    for b in range(B):
        xTb = np.ascontiguousarray(x[b].T)
        for h in range(2):
            qcols = np.concatenate(
                [np.arange(g * 256, (g + 1) * 256) for g in G[h]])
            in_maps.append(dict(
                xqT=np.ascontiguousarray(xTb[:, qcols]),
                xT=xTb,
                xn=x[b],
                wqT=wqT, wkT=wkT, wvT=wvT,
                masks=masks_h[h],
                ones=ones,
            ))

    nc = _get_nc()
    res = run_bass_kernel_spmd(nc, in_maps, core_ids=list(range(NCORES)),
                               trace=bool(os.environ.get("KERNEL_TRACE")))
    if os.environ.get("KERNEL_TRACE"):
        _NC_CACHE["last_results"] = res

    out = np.empty((B, S, DOUT), dtype=np.float32)
    for b in range(B):
        for h in range(2):
            oT = res.results[b * 2 + h]["outT"]
            for s2, g in enumerate(G[h]):
                out[b, g * 256:(g + 1) * 256, :] = \
                    oT[:, s2 * 256:(s2 + 1) * 256].T
    return out


# revision 18
# speedup vs baseline: 1.1560x; 1.0302x over previous
"""Causal attention (B=4, S=2048, D=1024, single 1024-dim head) on 8 TRN2 cores.

Sharding: data-parallel over batch (4) x 2-way causal-balanced query split
(zigzag 256-row query blocks: core A gets global blocks {0,3,4,7}, core B
{1,2,5,6}).  Every core runs the same SPMD program over 4 query slots with
k-chunk counts {4,8,12,16}; causality differences between the cores are
expressed purely through per-core input data (gathered q columns + mask
tiles), never through the instruction stream.

Device algorithm (transposed layouts throughout so every matmul contracts
over the partition dim with naturally-DMA-able operands):
  kT = (Wk xT)                   [dout, 2048]
  qT = (Wq xT_gathered)          [dout, 1024]
  per q-slot (256 cols), per k-chunk (128 rows):
      sT   = kT_chunk^T qT_slot  [128k, 256q]   (PSUM, 8 dout-chunk matmuls)
      expT = exp(sT/32)          (ACT, PSUM->SBUF; scores are O(+-8) so no
                                  max-subtraction is needed)
      mask-multiply (DVE) for the last 4 chunks of the slot (host tiles)
      dacc += expT               (DVE; softmax denominator partial sums)
      ctxU[d] += xn_chunk[:,d]^T expT   (PSUM accumulate: ctx = attn @ x,
                                  using attn@x@WvT == attn@(x WvT) assoc.)
  denom per slot: ones^T dacc (matmul) -> reciprocal -> broadcast matmul
                  -> recip row stored in bcsAll (normalization is linear, so
                  it is deferred to the phase-3 PSUM evacuation multiply)
  outT = (WvT^T ctxU) * bcsAll   [dout, 1024]
All matmuls run as float32r (fp32 storage, ~tf32 precision, 4x the fp32
matmul rate).
"""

import os
import sys

sys.path.insert(0, "/opt/trn_rl_repo")

import numpy as np

B, S, DIN, DOUT = 4, 2048, 1024, 1024
P = 128
NQ = 1024  # q rows per core
ND = DIN // P
NO = DOUT // P
NCORES = 8
G = [[0, 3, 4, 7], [1, 2, 5, 6]]  # global 256-row q-block per (core-half, slot)
L = [4, 8, 12, 16]  # k-chunks processed per slot (uniform across cores)

_NC_CACHE = {}


def _build_nc():
    import concourse.mybir as mybir
    import concourse.tile as tile
    from concourse import bacc
    from contextlib import ExitStack

    f32 = mybir.dt.float32
    f32r = mybir.dt.float32r
    EXP = mybir.ActivationFunctionType.Exp

    nc = bacc.Bacc("TRN2", target_bir_lowering=False, debug=False,
                   num_devices=NCORES)

    xqT_d = nc.dram_tensor("xqT", [DIN, NQ], f32, kind="ExternalInput").ap()
    xT_d = nc.dram_tensor("xT", [DIN, S], f32, kind="ExternalInput").ap()
    xn_d = nc.dram_tensor("xn", [S, DIN], f32, kind="ExternalInput").ap()
    wqT_d = nc.dram_tensor("wqT", [DIN, DOUT], f32, kind="ExternalInput").ap()
    wkT_d = nc.dram_tensor("wkT", [DIN, DOUT], f32, kind="ExternalInput").ap()
    wvT_d = nc.dram_tensor("wvT", [DIN, DOUT], f32, kind="ExternalInput").ap()
    masks_d = nc.dram_tensor("masks", [P, 16 * 256], f32, kind="ExternalInput").ap()
    ones_d = nc.dram_tensor("ones", [P, 160], f32, kind="ExternalInput").ap()
    outT_d = nc.dram_tensor("outT", [DOUT, NQ], f32, kind="ExternalOutput").ap()

    with tile.TileContext(nc) as tc:
        with ExitStack() as es:
            qT_pool = es.enter_context(tc.tile_pool(name="qTp", bufs=1))
            kT_pool = es.enter_context(tc.tile_pool(name="kTp", bufs=1))
            ctx_pool = es.enter_context(tc.tile_pool(name="ctxp", bufs=1))
            cst_pool = es.enter_context(tc.tile_pool(name="cst", bufs=1))

            qT = [qT_pool.tile([P, NQ], f32r, name=f"qT{o}", tag=f"qT{o}")
                  for o in range(NO)]
            kT = [kT_pool.tile([P, S], f32r, name=f"kT{o}", tag=f"kT{o}")
                  for o in range(NO)]
            onesT = cst_pool.tile([P, 160], f32r, name="onesT", tag="onesT")
            nc.sync.dma_start(onesT[:], ones_d[:].bitcast(f32r))
            ones_col = onesT[:, 0:1]      # [128, 1]
            ones_row = onesT[0:1, 32:160]  # [1, 128]
            # per-slot softmax 1/denominator rows, partition-broadcast
            bcsAll = cst_pool.tile([P, NQ], f32, name="bcsAll", tag="bcsAll")

            # ---------------- phase 1: k/q projections ----------------
            with tc.tile_pool(name="xs", bufs=10) as x_pool, \
                 tc.tile_pool(name="ws", bufs=8) as w_pool, \
                 tc.tile_pool(name="pps", bufs=5, space="PSUM") as proj_ps:
                # PE warmup during the initial DMA head: harmless matmuls on
                # the (tiny, loaded-first) ones tile keep the HAM clock gate
                # from idling while the first x/W tiles stream in.
                wu = proj_ps.tile([P, 128], f32, name="wu", tag="wu", bufs=1)
                for r in range(36):
                    nc.tensor.matmul(wu[:], onesT[:, 0:128], onesT[:, 0:128],
                                     start=True, stop=True,
                                     skip_group_check=True)

                # 1a: kT = Wk @ x^T, all 2048 keys in two column halves.
                # d-outer loop with 4 concurrent PSUM chains -> each weight
                # slice (lhsT) is loaded once per 4 matmuls.
                wks = []
                for d in range(ND):
                    wk = w_pool.tile([P, DOUT], f32r, name=f"wk{d}", tag="ws")
                    nc.sync.dma_start(
                        wk[:], wkT_d[d * P:(d + 1) * P, :].bitcast(f32r))
                    wks.append(wk)
                for half in range(2):
                    xhs = []
                    for d in range(ND):
                        xh = x_pool.tile([P, 1024], f32r, name=f"xh{half}_{d}",
                                         tag="xs")
                        nc.sync.dma_start(
                            xh[:],
                            xT_d[d * P:(d + 1) * P,
                                 half * 1024:(half + 1) * 1024].bitcast(f32r))
                        xhs.append(xh)
                    for o in range(NO):
                        pos = [proj_ps.tile([P, 512], f32, name=f"pok{kp}",
                                            tag="po") for kp in range(2)]
                        for d in range(ND):
                            for kp in range(2):
                                nc.tensor.matmul(
                                    pos[kp][:],
                                    wks[d][:, o * P:(o + 1) * P],
                                    xhs[d][:, kp * 512:(kp + 1) * 512],
                                    start=(d == 0), stop=(d == ND - 1))
                        for kp in range(2):
                            col = half * 1024 + kp * 512
                            nc.vector.tensor_copy(kT[o][:, col:col + 512],
                                                  pos[kp][:])

                # 1b: qT = Wq @ x^T (gathered q cols)
                xqs = []
                for d in range(ND):
                    xq = x_pool.tile([P, NQ], f32r, name=f"xq{d}", tag="xs")
                    nc.sync.dma_start(
                        xq[:], xqT_d[d * P:(d + 1) * P, :].bitcast(f32r))
                    xqs.append(xq)
                wqs = []
                for d in range(ND):
                    wq = w_pool.tile([P, DOUT], f32r, name=f"wq{d}", tag="ws")
                    nc.sync.dma_start(
                        wq[:], wqT_d[d * P:(d + 1) * P, :].bitcast(f32r))
                    wqs.append(wq)
                for o in range(NO):
                    pos = [proj_ps.tile([P, 512], f32, name=f"poq{h}",
                                        tag="po") for h in range(2)]
                    for d in range(ND):
                        for h in range(2):
                            nc.tensor.matmul(
                                pos[h][:],
                                wqs[d][:, o * P:(o + 1) * P],
                                xqs[d][:, h * 512:(h + 1) * 512],
                                start=(d == 0), stop=(d == ND - 1))
                    for h in range(2):
                        nc.vector.tensor_copy(qT[o][:, h * 512:(h + 1) * 512],
                                              pos[h][:])

            # ---------------- phase 2: attention ----------------
            with tc.tile_pool(name="msk", bufs=1) as msk_pool, \
                 tc.tile_pool(name="xnp", bufs=6) as xn_pool, \
                 tc.tile_pool(name="exq", bufs=4) as exp_pool, \
                 tc.tile_pool(name="dac", bufs=2) as dacc_pool, \
                 tc.tile_pool(name="sml", bufs=2) as small_pool, \
                 tc.tile_pool(name="sps", bufs=2, space="PSUM") as sT_ps, \
                 tc.tile_pool(name="cps", bufs=4, space="PSUM") as ctx_ps, \
                 tc.tile_pool(name="dps", bufs=1, space="PSUM") as dn_ps:
                maskT = msk_pool.tile([P, 16 * 256], f32r, name="maskT",
                                      tag="maskT")
                nc.sync.dma_start(maskT[:], masks_d[:].bitcast(f32r))
                ctxN = [ctx_pool.tile([P, NQ], f32r, name=f"ctx{d}",
                                      tag=f"ctx{d}") for d in range(ND)]
                for s in (3, 2, 1, 0):
                    q0 = s * 256
                    cps = [ctx_ps.tile([P, 512], f32, name=f"cps{s}_{i}",
                                       tag="cps") for i in range(4)]
                    dacc = dacc_pool.tile([P, 256], f32r, name="dacc", tag="dacc")
                    for c in range(L[s]):
                        st = sT_ps.tile([P, 256], f32, name="st", tag="st")
                        for o in range(NO):
                            nc.tensor.matmul(
                                st[:],
                                kT[o][:, c * P:(c + 1) * P],
                                qT[o][:, q0:q0 + 256],
                                start=(o == 0), stop=(o == NO - 1))
                        et = exp_pool.tile([P, 256], f32r, name="et", tag="et")
                        nc.scalar.activation(et[:], st[:], EXP, scale=1.0 / 32.0)
                        if c >= L[s] - 4:
                            m = 4 * s + (c - (L[s] - 4))
                            et2 = exp_pool.tile([P, 256], f32r, name="et2",
                                                tag="et2")
                            nc.vector.tensor_mul(
                                et2[:], et[:], maskT[:, m * 256:(m + 1) * 256])
                            et = et2
                        if c == 0:
                            nc.vector.tensor_copy(dacc[:], et[:])
                        else:
                            nc.vector.tensor_add(dacc[:], dacc[:], et[:])
                        xnt = xn_pool.tile([P, DIN], f32r, name="xnt", tag="xnt")
                        nc.sync.dma_start(
                            xnt[:], xn_d[c * P:(c + 1) * P, :].bitcast(f32r))
                        # Two accumulators share each PSUM bank; start=True
                        # clears has_written for the WHOLE bank, so only the
                        # first (even-d) group may use it.  The odd-d group
                        # initializes via start=False (bits are clear after
                        # the even start, so its first matmul overwrites),
                        # which requires the c==0 matmuls to execute in d
                        # order -> pin them with tile_critical.
                        if c == 0:
                            with tc.tile_critical():
                                for d in range(ND):
                                    acc = cps[d // 2][:, (d % 2) * 256:
                                                      (d % 2) * 256 + 256]
                                    nc.tensor.matmul(
                                        acc, xnt[:, d * P:(d + 1) * P], et[:],
                                        start=(d % 2 == 0),
                                        stop=(L[s] == 1))
                        else:
                            for d in range(ND):
                                acc = cps[d // 2][:, (d % 2) * 256:
                                                  (d % 2) * 256 + 256]
                                nc.tensor.matmul(
                                    acc, xnt[:, d * P:(d + 1) * P], et[:],
                                    start=False, stop=(c == L[s] - 1))
                    # Evacuate the ctx accumulators with plain copies (frees
                    # the PSUM banks for the next slot without waiting on the
                    # denominator chain).
                    for d in range(ND):
                        src = cps[d // 2][:, (d % 2) * 256:(d % 2) * 256 + 256]
                        nc.vector.tensor_copy(ctxN[d][:, q0:q0 + 256], src)
                    # softmax denominator -> reciprocal -> partition-broadcast
                    dsum = dn_ps.tile([1, 256], f32, name="dsum", tag="dsum")
                    nc.tensor.matmul(dsum[:], ones_col, dacc[:],
                                     start=True, stop=True)
                    rec = small_pool.tile([1, 256], f32r, name="rec", tag="rec")
                    with nc.allow_low_precision(
                            reason="f32r reciprocal feeds f32r bcast matmul"):
                        nc.vector.reciprocal(rec[:], dsum[:])
                    bc = dn_ps.tile([P, 256], f32, name="bc", tag="bc")
                    nc.tensor.matmul(bc[:], ones_row, rec[:],
                                     start=True, stop=True)
                    nc.vector.tensor_copy(bcsAll[:, q0:q0 + 256], bc[:])

            # ------- phase 3: out^T = (Wv ctx^T) * (1/denominator) -------
            # Wv tiles reuse the qT pool slots (qT is dead once the last
            # slot's score matmuls have read it), so the Wv DMAs can issue
            # during the phase-2 tail without extra SBUF.
            with tc.tile_pool(name="obp", bufs=4) as out_pool, \
                 tc.tile_pool(name="ops", bufs=5, space="PSUM") as out_ps:
                wvs = []
                for d in range(ND):
                    wv = qT_pool.tile([P, DOUT], f32r, name=f"wv{d}",
                                      tag=f"qT{d}")
                    nc.sync.dma_start(
                        wv[:], wvT_d[d * P:(d + 1) * P, :].bitcast(f32r))
                    wvs.append(wv)
                for o in range(NO):
                    pos = [out_ps.tile([P, 512], f32, name=f"poo{qp}",
                                       tag="poo") for qp in range(2)]
                    for d in range(ND):
                        for qp in range(2):
                            nc.tensor.matmul(
                                pos[qp][:],
                                wvs[d][:, o * P:(o + 1) * P],
                                ctxN[d][:, qp * 512:(qp + 1) * 512],
                                start=(d == 0), stop=(d == ND - 1))
                    for qp in range(2):
                        ob = out_pool.tile([P, 512], f32, name="ob", tag="ob")
                        nc.vector.tensor_mul(
                            ob[:], pos[qp][:],
                            bcsAll[:, qp * 512:(qp + 1) * 512])
                        nc.sync.dma_start(
                            outT_d[o * P:(o + 1) * P, qp * 512:(qp + 1) * 512],
                            ob[:])

    nc.compile()
    return nc


def _get_nc():
    if "nc" not in _NC_CACHE:
        _NC_CACHE["nc"] = _build_nc()
    return _NC_CACHE["nc"]


def _make_masks(h):
    """[128, 16*256] mask tile row: 1.0 where key 128c+p <= query 256g+j."""
    mk = np.zeros((P, 16 * 256), dtype=np.float32)
    p = np.arange(P)[:, None]
    j = np.arange(256)[None, :]
    for s in range(4):
        g = G[h][s]
        for m in range(4):
            c = L[s] - 4 + m
            mk[:, (4 * s + m) * 256:(4 * s + m + 1) * 256] = (
                (128 * c + p) <= (256 * g + j)).astype(np.float32)
    return mk


def kernel(x, W_q, W_k, W_v):
    from concourse.bass_utils import run_bass_kernel_spmd

    x = np.ascontiguousarray(np.asarray(x, dtype=np.float32))
    wqT = np.ascontiguousarray(np.asarray(W_q, dtype=np.float32).T)
    wkT = np.ascontiguousarray(np.asarray(W_k, dtype=np.float32).T)
    wvT = np.ascontiguousarray(np.asarray(W_v, dtype=np.float32).T)

    ones = np.zeros((P, 160), dtype=np.float32)
    ones[:, 0] = 1.0
    ones[0, 32:160] = 1.0
    masks_h = [_make_masks(0), _make_masks(1)]

    in_maps = []
    for b in range(B):
        xTb = np.ascontiguousarray(x[b].T)
        for h in range(2):
            qcols = np.concatenate(
                [np.arange(g * 256, (g + 1) * 256) for g in G[h]])
            in_maps.append(dict(
                xqT=np.ascontiguousarray(xTb[:, qcols]),
                xT=xTb,
                xn=x[b],
                wqT=wqT, wkT=wkT, wvT=wvT,
                masks=masks_h[h],
                ones=ones,
            ))

    nc = _get_nc()
    res = run_bass_kernel_spmd(nc, in_maps, core_ids=list(range(NCORES)),
                               trace=bool(os.environ.get("KERNEL_TRACE")))
    if os.environ.get("KERNEL_TRACE"):
        _NC_CACHE["last_results"] = res

    out = np.empty((B, S, DOUT), dtype=np.float32)
    for b in range(B):
        for h in range(2):
            oT = res.results[b * 2 + h]["outT"]
            for s2, g in enumerate(G[h]):
                out[b, g * 256:(g + 1) * 256, :] = \
                    oT[:, s2 * 256:(s2 + 1) * 256].T
    return out


# revision 19
# speedup vs baseline: 1.2932x; 1.1187x over previous
"""Causal attention (B=4, S=2048, D=1024, single 1024-dim head) on 8 TRN2 cores.

Sharding: data-parallel over batch (4) x 2-way causal-balanced query split
(zigzag 256-row query blocks: core A gets global blocks {0,3,4,7}, core B
{1,2,5,6}).  Every core runs the same SPMD program over 4 query slots with
k-chunk counts {4,8,12,16}; causality differences between the cores are
expressed purely through per-core input data (gathered q columns + mask
tiles), never through the instruction stream.

Device algorithm (transposed layouts throughout so every matmul contracts
over the partition dim with naturally-DMA-able operands):
  kT = (Wk xT)                   [dout, 2048]
  qT = (Wq xT_gathered)          [dout, 1024]
  per q-slot (256 cols), per k-chunk (128 rows):
      sT   = kT_chunk^T qT_slot  [128k, 256q]   (PSUM, 8 dout-chunk matmuls)
      expT = exp(sT/32)          (ACT, PSUM->SBUF fp16; scores are O(+-8) so
                                  no max-subtraction is needed)
      mask-multiply (DVE) for the last 4 chunks of the slot (host tiles)
      dsum += ones^T expT        (PE, fp32 PSUM accumulation across chunks)
      ctxU[d] += xn_chunk[:,d]^T expT   (PSUM accumulate: ctx = attn @ x,
                                  using attn@x@WvT == attn@(x WvT) assoc.)
  per slot: reciprocal(dsum) -> broadcast matmul -> bcsAll row (the softmax
  normalization is linear, so it is deferred to the phase-3 evacuation mul)
  outT = (WvT^T ctxU) * bcsAll   [dout, 1024]
Matmul operands are fp16 (host-converted); accumulation PSUM is fp32, the
softmax denominator path is fp32, output is fp32.
"""

import os
import sys

sys.path.insert(0, "/opt/trn_rl_repo")

import numpy as np

B, S, DIN, DOUT = 4, 2048, 1024, 1024
P = 128
NQ = 1024  # q rows per core
ND = DIN // P
NO = DOUT // P
NK = S // P  # 16 key chunks
NCORES = 8
G = [[0, 3, 4, 7], [1, 2, 5, 6]]  # global 256-row q-block per (core-half, slot)
L = [4, 8, 12, 16]  # k-chunks processed per slot (uniform across cores)

_NC_CACHE = {}


def _build_nc():
    import concourse.mybir as mybir
    import concourse.tile as tile
    from concourse import bacc
    from contextlib import ExitStack

    f32 = mybir.dt.float32
    f16 = mybir.dt.float16
    EXP = mybir.ActivationFunctionType.Exp

    nc = bacc.Bacc("TRN2", target_bir_lowering=False, debug=False,
                   num_devices=NCORES)

    xqT_d = nc.dram_tensor("xqT", [DIN, NQ], f16, kind="ExternalInput").ap()
    xT_d = nc.dram_tensor("xT", [DIN, S], f16, kind="ExternalInput").ap()
    xn_d = nc.dram_tensor("xn", [S, DIN], f16, kind="ExternalInput").ap()
    wqT_d = nc.dram_tensor("wqT", [DIN, DOUT], f16, kind="ExternalInput").ap()
    wkT_d = nc.dram_tensor("wkT", [DIN, DOUT], f16, kind="ExternalInput").ap()
    wvT_d = nc.dram_tensor("wvT", [DIN, DOUT], f16, kind="ExternalInput").ap()
    masks_d = nc.dram_tensor("masks", [P, 16 * 256], f16, kind="ExternalInput").ap()
    ones_d = nc.dram_tensor("ones", [P, 160], f16, kind="ExternalInput").ap()
    outT_d = nc.dram_tensor("outT", [DOUT, NQ], f32, kind="ExternalOutput").ap()

    with tile.TileContext(nc) as tc:
        with ExitStack() as es:
            qT_pool = es.enter_context(tc.tile_pool(name="qTp", bufs=1))
            kT_pool = es.enter_context(tc.tile_pool(name="kTp", bufs=1))
            ctx_pool = es.enter_context(tc.tile_pool(name="ctxp", bufs=1))
            cst_pool = es.enter_context(tc.tile_pool(name="cst", bufs=1))
            xn_pool = es.enter_context(tc.tile_pool(name="xnp", bufs=1))

            qT = [qT_pool.tile([P, NQ], f16, name=f"qT{o}", tag=f"qT{o}")
                  for o in range(NO)]
            kT = [kT_pool.tile([P, S], f16, name=f"kT{o}", tag=f"kT{o}")
                  for o in range(NO)]
            onesT = cst_pool.tile([P, 160], f16, name="onesT", tag="onesT")
            nc.sync.dma_start(onesT[:], ones_d[:])
            ones_col = onesT[:, 0:1]      # [128, 1]
            ones_row = onesT[0:1, 32:160]  # [1, 128]
            # per-slot softmax 1/denominator rows, partition-broadcast (fp32)
            bcsAll = cst_pool.tile([P, NQ], f32, name="bcsAll", tag="bcsAll")
            maskT = cst_pool.tile([P, 16 * 256], f16, name="maskT", tag="maskT")
            ctxN = [ctx_pool.tile([P, NQ], f16, name=f"ctx{d}", tag=f"ctx{d}")
                    for d in range(ND)]
            # x rows (AV stationary operand): resident for all of phase 2
            xn16 = [xn_pool.tile([P, DIN], f16, name=f"xn{c}", tag=f"xn{c}")
                    for c in range(NK)]

            # ---------------- phase 1: k/q projections ----------------
            with tc.tile_pool(name="xs", bufs=16) as x_pool, \
                 tc.tile_pool(name="ws", bufs=10) as w_pool, \
                 tc.tile_pool(name="pps", bufs=5, space="PSUM") as proj_ps:
                # PE warmup during the initial DMA head: harmless matmuls on
                # the (tiny, loaded-first) ones tile keep the HAM clock gate
                # from idling while the first x/W tiles stream in.
                wu = proj_ps.tile([P, 128], f32, name="wu", tag="wu", bufs=1)
                for r in range(64):
                    nc.tensor.matmul(wu[:], onesT[:, 0:128], onesT[:, 0:128],
                                     start=True, stop=True,
                                     skip_group_check=True)

                # 1a: kT = Wk @ x^T, all 2048 keys in two column halves.
                # d-outer loop with 2 concurrent PSUM chains -> each weight
                # slice (lhsT) is loaded once per 2 matmuls.
                wks = []
                for d in range(ND):
                    wk = w_pool.tile([P, DOUT], f16, name=f"wk{d}", tag="ws")
                    nc.sync.dma_start(wk[:], wkT_d[d * P:(d + 1) * P, :])
                    wks.append(wk)
                xhs_all = {}
                for half in range(2):
                    for d in range(ND):
                        xh = x_pool.tile([P, 1024], f16, name=f"xh{half}_{d}",
                                         tag="xs")
                        nc.sync.dma_start(
                            xh[:], xT_d[d * P:(d + 1) * P,
                                        half * 1024:(half + 1) * 1024])
                        xhs_all[(half, d)] = xh
                # attention stationary x rows + masks stream in behind the
                # projection operands, well before phase 2 needs them
                for c in range(NK):
                    nc.sync.dma_start(xn16[c][:], xn_d[c * P:(c + 1) * P, :])
                nc.sync.dma_start(maskT[:], masks_d[:])

                for half in range(2):
                    xhs = [xhs_all[(half, d)] for d in range(ND)]
                    for o in range(NO):
                        pos = [proj_ps.tile([P, 512], f32, name=f"pok{kp}",
                                            tag="po") for kp in range(2)]
                        for d in range(ND):
                            for kp in range(2):
                                nc.tensor.matmul(
                                    pos[kp][:],
                                    wks[d][:, o * P:(o + 1) * P],
                                    xhs[d][:, kp * 512:(kp + 1) * 512],
                                    start=(d == 0), stop=(d == ND - 1))
                        for kp in range(2):
                            col = half * 1024 + kp * 512
                            nc.vector.tensor_copy(kT[o][:, col:col + 512],
                                                  pos[kp][:])

                # 1b: qT = Wq @ x^T (gathered q cols)
                xqs = []
                for d in range(ND):
                    xq = x_pool.tile([P, NQ], f16, name=f"xq{d}", tag="xs")
                    nc.sync.dma_start(xq[:], xqT_d[d * P:(d + 1) * P, :])
                    xqs.append(xq)
                wqs = []
                for d in range(ND):
                    wq = w_pool.tile([P, DOUT], f16, name=f"wq{d}", tag="ws")
                    nc.sync.dma_start(wq[:], wqT_d[d * P:(d + 1) * P, :])
                    wqs.append(wq)
                for o in range(NO):
                    pos = [proj_ps.tile([P, 512], f32, name=f"poq{h}",
                                        tag="po") for h in range(2)]
                    for d in range(ND):
                        for h in range(2):
                            nc.tensor.matmul(
                                pos[h][:],
                                wqs[d][:, o * P:(o + 1) * P],
                                xqs[d][:, h * 512:(h + 1) * 512],
                                start=(d == 0), stop=(d == ND - 1))
                    for h in range(2):
                        nc.vector.tensor_copy(qT[o][:, h * 512:(h + 1) * 512],
                                              pos[h][:])

            # ---------------- phase 2: attention ----------------
            with tc.tile_pool(name="exq", bufs=5) as exp_pool, \
                 tc.tile_pool(name="sml", bufs=2) as small_pool, \
                 tc.tile_pool(name="sps", bufs=2, space="PSUM") as sT_ps, \
                 tc.tile_pool(name="cps", bufs=4, space="PSUM") as ctx_ps, \
                 tc.tile_pool(name="dps", bufs=1, space="PSUM") as dn_ps:
                for s in (3, 2, 1, 0):
                    q0 = s * 256
                    cps = [ctx_ps.tile([P, 512], f32, name=f"cps{s}_{i}",
                                       tag="cps") for i in range(4)]
                    dsum = dn_ps.tile([1, 256], f32, name=f"dsum{s}",
                                      tag="dsum")

                    def st_chunk(c):
                        st = sT_ps.tile([P, 256], f32, name="st", tag="st")
                        for o in range(NO):
                            nc.tensor.matmul(
                                st[:],
                                kT[o][:, c * P:(c + 1) * P],
                                qT[o][:, q0:q0 + 256],
                                start=(o == 0), stop=(o == NO - 1))
                        et = exp_pool.tile([P, 256], f16, name="et", tag="et")
                        nc.scalar.activation(et[:], st[:], EXP, scale=1.0 / 32.0)
                        if c >= L[s] - 4:
                            m = 4 * s + (c - (L[s] - 4))
                            et2 = exp_pool.tile([P, 256], f16, name="et2",
                                                tag="et2")
                            nc.vector.tensor_mul(
                                et2[:], et[:], maskT[:, m * 256:(m + 1) * 256])
                            et = et2
                        return et

                    def av_chunk(c, et):
                        # softmax denominator: fp32 PSUM row accumulated on PE
                        nc.tensor.matmul(dsum[:], ones_col, et[:],
                                         start=(c == 0), stop=(c == L[s] - 1))
                        # Two ctx accumulators share each PSUM bank;
                        # start=True clears has_written for the WHOLE bank,
                        # so only the first (even-d) group may use it.  The
                        # odd-d group initializes via start=False (bits are
                        # clear after the even start, so its first matmul
                        # overwrites), which requires the c==0 matmuls to
                        # execute in d order -> pin them with tile_critical.
                        if c == 0:
                            with tc.tile_critical():
                                for d in range(ND):
                                    acc = cps[d // 2][:, (d % 2) * 256:
                                                      (d % 2) * 256 + 256]
                                    nc.tensor.matmul(
                                        acc, xn16[c][:, d * P:(d + 1) * P],
                                        et[:], start=(d % 2 == 0),
                                        stop=(L[s] == 1))
                        else:
                            for d in range(ND):
                                acc = cps[d // 2][:, (d % 2) * 256:
                                                  (d % 2) * 256 + 256]
                                nc.tensor.matmul(
                                    acc, xn16[c][:, d * P:(d + 1) * P], et[:],
                                    start=False, stop=(c == L[s] - 1))

                    # software pipeline: score chain for chunk c+1 is emitted
                    # before the AV matmuls of chunk c, so the PE always has
                    # score work while ACT computes exp / PSUM banks recycle
                    ets = {0: st_chunk(0)}
                    for c in range(L[s]):
                        if c + 1 < L[s]:
                            ets[c + 1] = st_chunk(c + 1)
                        av_chunk(c, ets.pop(c))

                    # evacuate ctx accumulators with plain copies (frees the
                    # PSUM banks without waiting on the denominator chain)
                    for d in range(ND):
                        src = cps[d // 2][:, (d % 2) * 256:(d % 2) * 256 + 256]
                        nc.vector.tensor_copy(ctxN[d][:, q0:q0 + 256], src)
                    # reciprocal -> partition-broadcast -> bcsAll row
                    rec = small_pool.tile([1, 256], f16, name="rec", tag="rec")
                    with nc.allow_low_precision(
                            reason="fp16 recip feeds fp16 bcast matmul"):
                        nc.vector.reciprocal(rec[:], dsum[:])
                    bc = dn_ps.tile([P, 256], f32, name="bc", tag="bc")
                    nc.tensor.matmul(bc[:], ones_row, rec[:],
                                     start=True, stop=True)
                    nc.vector.tensor_copy(bcsAll[:, q0:q0 + 256], bc[:])

            # ------- phase 3: out^T = (Wv ctx^T) * (1/denominator) -------
            # Wv tiles reuse the qT pool slots (qT is dead once the last
            # slot's score matmuls have read it), so the Wv DMAs can issue
            # during the phase-2 tail without extra SBUF.
            with tc.tile_pool(name="obp", bufs=4) as out_pool, \
                 tc.tile_pool(name="ops", bufs=5, space="PSUM") as out_ps:
                wvs = []
                for d in range(ND):
                    wv = qT_pool.tile([P, DOUT], f16, name=f"wv{d}",
                                      tag=f"qT{d}")
                    nc.sync.dma_start(wv[:], wvT_d[d * P:(d + 1) * P, :])
                    wvs.append(wv)
                for o in range(NO):
                    pos = [out_ps.tile([P, 512], f32, name=f"poo{qp}",
                                       tag="poo") for qp in range(2)]
                    for d in range(ND):
                        for qp in range(2):
                            nc.tensor.matmul(
                                pos[qp][:],
                                wvs[d][:, o * P:(o + 1) * P],
                                ctxN[d][:, qp * 512:(qp + 1) * 512],
                                start=(d == 0), stop=(d == ND - 1))
                    for qp in range(2):
                        ob = out_pool.tile([P, 512], f32, name="ob", tag="ob")
                        nc.vector.tensor_mul(
                            ob[:], pos[qp][:],
                            bcsAll[:, qp * 512:(qp + 1) * 512])
                        nc.sync.dma_start(
                            outT_d[o * P:(o + 1) * P, qp * 512:(qp + 1) * 512],
                            ob[:])

    nc.compile()
    return nc


def _get_nc():
    if "nc" not in _NC_CACHE:
        _NC_CACHE["nc"] = _build_nc()
    return _NC_CACHE["nc"]


def _make_masks(h):
    """[128, 16*256] mask tile row: 1.0 where key 128c+p <= query 256g+j."""
    mk = np.zeros((P, 16 * 256), dtype=np.float16)
    p = np.arange(P)[:, None]
    j = np.arange(256)[None, :]
    for s in range(4):
        g = G[h][s]
        for m in range(4):
            c = L[s] - 4 + m
            mk[:, (4 * s + m) * 256:(4 * s + m + 1) * 256] = (
                (128 * c + p) <= (256 * g + j)).astype(np.float16)
    return mk


def kernel(x, W_q, W_k, W_v):
    from concourse.bass_utils import run_bass_kernel_spmd

    x = np.asarray(x, dtype=np.float32)
    x16 = x.astype(np.float16)
    wqT = np.ascontiguousarray(np.asarray(W_q, dtype=np.float32).T
                               .astype(np.float16))
    wkT = np.ascontiguousarray(np.asarray(W_k, dtype=np.float32).T
                               .astype(np.float16))
    wvT = np.ascontiguousarray(np.asarray(W_v, dtype=np.float32).T
                               .astype(np.float16))

    ones = np.zeros((P, 160), dtype=np.float16)
    ones[:, 0] = 1.0
    ones[0, 32:160] = 1.0
    masks_h = [_make_masks(0), _make_masks(1)]

    in_maps = []
    for b in range(B):
        xTb = np.ascontiguousarray(x16[b].T)
        for h in range(2):
            qcols = np.concatenate(
                [np.arange(g * 256, (g + 1) * 256) for g in G[h]])
            in_maps.append(dict(
                xqT=np.ascontiguousarray(xTb[:, qcols]),
                xT=xTb,
                xn=np.ascontiguousarray(x16[b]),
                wqT=wqT, wkT=wkT, wvT=wvT,
                masks=masks_h[h],
                ones=ones,
            ))

    nc = _get_nc()
    res = run_bass_kernel_spmd(nc, in_maps, core_ids=list(range(NCORES)),
                               trace=bool(os.environ.get("KERNEL_TRACE")))
    if os.environ.get("KERNEL_TRACE"):
        _NC_CACHE["last_results"] = res

    out = np.empty((B, S, DOUT), dtype=np.float32)
    for b in range(B):
        for h in range(2):
            oT = res.results[b * 2 + h]["outT"]
            for s2, g in enumerate(G[h]):
                out[b, g * 256:(g + 1) * 256, :] = \
                    oT[:, s2 * 256:(s2 + 1) * 256].T
    return out


# revision 23
# speedup vs baseline: 1.2940x; 1.0007x over previous
"""Causal attention (B=4, S=2048, D=1024, single 1024-dim head) on 8 TRN2 cores.

Sharding: data-parallel over batch (4) x 2-way causal-balanced query split
(zigzag 256-row query blocks: core A gets global blocks {0,3,4,7}, core B
{1,2,5,6}).  Every core runs the same SPMD program over 4 query slots with
k-chunk counts {4,8,12,16}; causality differences between the cores are
expressed purely through per-core input data (gathered q columns + mask
tiles), never through the instruction stream.

Device algorithm (transposed layouts throughout so every matmul contracts
over the partition dim with naturally-DMA-able operands):
  kT = (Wk xT)                   [dout, 2048]
  qT = (Wq xT_gathered)          [dout, 1024]
  per q-slot (256 cols), per k-chunk (128 rows):
      sT   = kT_chunk^T qT_slot  [128k, 256q]   (PSUM, 8 dout-chunk matmuls)
      expT = exp(sT/32)          (ACT, PSUM->SBUF fp16; scores are O(+-8) so
                                  no max-subtraction is needed)
      mask-multiply (DVE) for the last 4 chunks of the slot (host tiles)
      dsum += ones^T expT        (PE, fp32 PSUM accumulation across chunks)
      ctxU[d] += xn_chunk[:,d]^T expT   (PSUM accumulate: ctx = attn @ x,
                                  using attn@x@WvT == attn@(x WvT) assoc.)
  per slot: reciprocal(dsum) -> broadcast matmul -> bcsAll row (the softmax
  normalization is linear, so it is deferred to the phase-3 evacuation mul)
  outT = (WvT^T ctxU) * bcsAll   [dout, 1024]
Matmul operands are fp16 (host-converted); accumulation PSUM is fp32, the
softmax denominator path is fp32, output is fp32.
"""

import os
import sys

sys.path.insert(0, "/opt/trn_rl_repo")

import numpy as np

B, S, DIN, DOUT = 4, 2048, 1024, 1024
P = 128
NQ = 1024  # q rows per core
ND = DIN // P
NO = DOUT // P
NK = S // P  # 16 key chunks
NCORES = 8
G = [[0, 3, 4, 7], [1, 2, 5, 6]]  # global 256-row q-block per (core-half, slot)
L = [4, 8, 12, 16]  # k-chunks processed per slot (uniform across cores)

_NC_CACHE = {}


def _build_nc():
    import concourse.mybir as mybir
    import concourse.tile as tile
    from concourse import bacc
    from contextlib import ExitStack

    f32 = mybir.dt.float32
    f16 = mybir.dt.float16
    EXP = mybir.ActivationFunctionType.Exp

    nc = bacc.Bacc("TRN2", target_bir_lowering=False, debug=False,
                   num_devices=NCORES)

    xqT_d = nc.dram_tensor("xqT", [DIN, NQ], f16, kind="ExternalInput").ap()
    xT_d = nc.dram_tensor("xT", [DIN, S], f16, kind="ExternalInput").ap()
    xn_d = nc.dram_tensor("xn", [S, DIN], f16, kind="ExternalInput").ap()
    wqT_d = nc.dram_tensor("wqT", [DIN, DOUT], f16, kind="ExternalInput").ap()
    wkT_d = nc.dram_tensor("wkT", [DIN, DOUT], f16, kind="ExternalInput").ap()
    wvT_d = nc.dram_tensor("wvT", [DIN, DOUT], f16, kind="ExternalInput").ap()
    masks_d = nc.dram_tensor("masks", [P, 16 * 256], f16, kind="ExternalInput").ap()
    ones_d = nc.dram_tensor("ones", [P, 160], f16, kind="ExternalInput").ap()
    outT_d = nc.dram_tensor("outT", [DOUT, NQ], f32, kind="ExternalOutput").ap()

    with tile.TileContext(nc) as tc:
        with ExitStack() as es:
            qT_pool = es.enter_context(tc.tile_pool(name="qTp", bufs=1))
            kT_pool = es.enter_context(tc.tile_pool(name="kTp", bufs=1))
            ctx_pool = es.enter_context(tc.tile_pool(name="ctxp", bufs=1))
            cst_pool = es.enter_context(tc.tile_pool(name="cst", bufs=1))
            xn_pool = es.enter_context(tc.tile_pool(name="xnp", bufs=1))

            qT = [qT_pool.tile([P, NQ], f16, name=f"qT{o}", tag=f"qT{o}")
                  for o in range(NO)]
            kT = [kT_pool.tile([P, S], f16, name=f"kT{o}", tag=f"kT{o}")
                  for o in range(NO)]
            onesT = cst_pool.tile([P, 160], f16, name="onesT", tag="onesT")
            nc.sync.dma_start(onesT[:], ones_d[:])
            ones_col = onesT[:, 0:1]      # [128, 1]
            ones_row = onesT[0:1, 32:160]  # [1, 128]
            # per-slot softmax 1/denominator rows, partition-broadcast (fp32)
            bcsAll = cst_pool.tile([P, NQ], f32, name="bcsAll", tag="bcsAll")
            maskT = cst_pool.tile([P, 16 * 256], f16, name="maskT", tag="maskT")
            ctxN = [ctx_pool.tile([P, NQ], f16, name=f"ctx{d}", tag=f"ctx{d}")
                    for d in range(ND)]
            # x rows (AV stationary operand): resident for all of phase 2
            xn16 = [xn_pool.tile([P, DIN], f16, name=f"xn{c}", tag=f"xn{c}")
                    for c in range(NK)]

            # ---------------- phase 1: k/q projections ----------------
            with tc.tile_pool(name="xs", bufs=16) as x_pool, \
                 tc.tile_pool(name="ws", bufs=10) as w_pool, \
                 tc.tile_pool(name="pps", bufs=5, space="PSUM") as proj_ps:
                # PE warmup during the initial DMA head: harmless matmuls on
                # the (tiny, loaded-first) ones tile keep the HAM clock gate
                # from idling while the first x/W tiles stream in.
                wu = proj_ps.tile([P, 128], f32, name="wu", tag="wu", bufs=1)
                for r in range(96):
                    nc.tensor.matmul(wu[:], onesT[:, 0:128], onesT[:, 0:128],
                                     start=True, stop=True,
                                     skip_group_check=True)

                # 1a: kT = Wk @ x^T, all 2048 keys in two column halves.
                # d-outer loop with 2 concurrent PSUM chains -> each weight
                # slice (lhsT) is loaded once per 2 matmuls.
                wks = []
                for d in range(ND):
                    wk = w_pool.tile([P, DOUT], f16, name=f"wk{d}", tag="ws")
                    nc.sync.dma_start(wk[:], wkT_d[d * P:(d + 1) * P, :])
                    wks.append(wk)
                xhs_all = {}
                for half in range(2):
                    for d in range(ND):
                        xh = x_pool.tile([P, 1024], f16, name=f"xh{half}_{d}",
                                         tag="xs")
                        nc.sync.dma_start(
                            xh[:], xT_d[d * P:(d + 1) * P,
                                        half * 1024:(half + 1) * 1024])
                        xhs_all[(half, d)] = xh
                # attention stationary x rows + masks stream in behind the
                # projection operands, well before phase 2 needs them
                for c in range(NK):
                    nc.sync.dma_start(xn16[c][:], xn_d[c * P:(c + 1) * P, :])
                nc.sync.dma_start(maskT[:], masks_d[:])

                for half in range(2):
                    xhs = [xhs_all[(half, d)] for d in range(ND)]
                    for o in range(NO):
                        pos = [proj_ps.tile([P, 512], f32, name=f"pok{kp}",
                                            tag="po") for kp in range(2)]
                        for d in range(ND):
                            for kp in range(2):
                                nc.tensor.matmul(
                                    pos[kp][:],
                                    wks[d][:, o * P:(o + 1) * P],
                                    xhs[d][:, kp * 512:(kp + 1) * 512],
                                    start=(d == 0), stop=(d == ND - 1))
                        for kp in range(2):
                            col = half * 1024 + kp * 512
                            nc.vector.tensor_copy(kT[o][:, col:col + 512],
                                                  pos[kp][:])

                # 1b: qT = Wq @ x^T (gathered q cols)
                xqs = []
                for d in range(ND):
                    xq = x_pool.tile([P, NQ], f16, name=f"xq{d}", tag="xs")
                    nc.sync.dma_start(xq[:], xqT_d[d * P:(d + 1) * P, :])
                    xqs.append(xq)
                wqs = []
                for d in range(ND):
                    wq = w_pool.tile([P, DOUT], f16, name=f"wq{d}", tag="ws")
                    nc.sync.dma_start(wq[:], wqT_d[d * P:(d + 1) * P, :])
                    wqs.append(wq)
                for o in range(NO):
                    pos = [proj_ps.tile([P, 512], f32, name=f"poq{h}",
                                        tag="po") for h in range(2)]
                    for d in range(ND):
                        for h in range(2):
                            nc.tensor.matmul(
                                pos[h][:],
                                wqs[d][:, o * P:(o + 1) * P],
                                xqs[d][:, h * 512:(h + 1) * 512],
                                start=(d == 0), stop=(d == ND - 1))
                    for h in range(2):
                        nc.vector.tensor_copy(qT[o][:, h * 512:(h + 1) * 512],
                                              pos[h][:])

            # ---------------- phase 2: attention ----------------
            recs = {}
            with tc.tile_pool(name="exq", bufs=5) as exp_pool, \
                 tc.tile_pool(name="sps", bufs=2, space="PSUM") as sT_ps, \
                 tc.tile_pool(name="cps", bufs=4, space="PSUM") as ctx_ps, \
                 tc.tile_pool(name="dps", bufs=2, space="PSUM") as dn_ps:
                for s in (3, 2, 1, 0):
                    q0 = s * 256
                    cps = [ctx_ps.tile([P, 512], f32, name=f"cps{s}_{i}",
                                       tag="cps") for i in range(4)]
                    dsum = dn_ps.tile([1, 256], f32, name=f"dsum{s}",
                                      tag="dsum")

                    def st_chunk(c):
                        st = sT_ps.tile([P, 256], f32, name="st", tag="st")
                        for o in range(NO):
                            nc.tensor.matmul(
                                st[:],
                                kT[o][:, c * P:(c + 1) * P],
                                qT[o][:, q0:q0 + 256],
                                start=(o == 0), stop=(o == NO - 1))
                        et = exp_pool.tile([P, 256], f16, name="et", tag="et")
                        nc.scalar.activation(et[:], st[:], EXP, scale=1.0 / 32.0)
                        if c >= L[s] - 4:
                            m = 4 * s + (c - (L[s] - 4))
                            et2 = exp_pool.tile([P, 256], f16, name="et2",
                                                tag="et2")
                            nc.vector.tensor_mul(
                                et2[:], et[:], maskT[:, m * 256:(m + 1) * 256])
                            et = et2
                        return et

                    def av_chunk(c, et):
                        # softmax denominator: fp32 PSUM row accumulated on PE
                        nc.tensor.matmul(dsum[:], ones_col, et[:],
                                         start=(c == 0), stop=(c == L[s] - 1))
                        # Two ctx accumulators share each PSUM bank;
                        # start=True clears has_written for the WHOLE bank,
                        # so only the first (even-d) group may use it.  The
                        # odd-d group initializes via start=False (bits are
                        # clear after the even start, so its first matmul
                        # overwrites), which requires the c==0 matmuls to
                        # execute in d order -> pin them with tile_critical.
                        if c == 0:
                            with tc.tile_critical():
                                for d in range(ND):
                                    acc = cps[d // 2][:, (d % 2) * 256:
                                                      (d % 2) * 256 + 256]
                                    nc.tensor.matmul(
                                        acc, xn16[c][:, d * P:(d + 1) * P],
                                        et[:], start=(d % 2 == 0),
                                        stop=(L[s] == 1))
                        else:
                            for d in range(ND):
                                acc = cps[d // 2][:, (d % 2) * 256:
                                                  (d % 2) * 256 + 256]
                                nc.tensor.matmul(
                                    acc, xn16[c][:, d * P:(d + 1) * P], et[:],
                                    start=False, stop=(c == L[s] - 1))

                    # software pipeline: score chain for chunk c+1 is emitted
                    # before the AV matmuls of chunk c, so the PE always has
                    # score work while ACT computes exp / PSUM banks recycle
                    ets = {0: st_chunk(0)}
                    for c in range(L[s]):
                        if c + 1 < L[s]:
                            ets[c + 1] = st_chunk(c + 1)
                        av_chunk(c, ets.pop(c))

                    # evacuate ctx accumulators with plain copies (frees the
                    # PSUM banks without waiting on the denominator chain)
                    for d in range(ND):
                        src = cps[d // 2][:, (d % 2) * 256:(d % 2) * 256 + 256]
                        nc.vector.tensor_copy(ctxN[d][:, q0:q0 + 256], src)
                    # reciprocal now; the partition-broadcast matmul is
                    # deferred to phase 3 so it never stalls the PE FIFO
                    # between slots
                    rec = cst_pool.tile([1, 256], f16, name=f"rec{s}",
                                        tag=f"rec{s}")
                    with nc.allow_low_precision(
                            reason="fp16 recip feeds fp16 bcast matmul"):
                        nc.vector.reciprocal(rec[:], dsum[:])
                    recs[s] = rec

            # ------- phase 3: out^T = (Wv ctx^T) * (1/denominator) -------
            # Wv tiles reuse the qT pool slots (qT is dead once the last
            # slot's score matmuls have read it), so the Wv DMAs can issue
            # during the phase-2 tail without extra SBUF.
            with tc.tile_pool(name="obp", bufs=4) as out_pool, \
                 tc.tile_pool(name="ops", bufs=5, space="PSUM") as out_ps:
                for s in range(4):
                    bc = out_ps.tile([P, 256], f32, name=f"bc{s}", tag="bc",
                                     bufs=2)
                    nc.tensor.matmul(bc[:], ones_row, recs[s][:],
                                     start=True, stop=True)
                    nc.vector.tensor_copy(bcsAll[:, s * 256:(s + 1) * 256],
                                          bc[:])
                wvs = []
                for d in range(ND):
                    wv = qT_pool.tile([P, DOUT], f16, name=f"wv{d}",
                                      tag=f"qT{d}")
                    nc.sync.dma_start(wv[:], wvT_d[d * P:(d + 1) * P, :])
                    wvs.append(wv)
                for o in range(NO):
                    pos = [out_ps.tile([P, 512], f32, name=f"poo{qp}",
                                       tag="poo") for qp in range(2)]
                    for d in range(ND):
                        for qp in range(2):
                            nc.tensor.matmul(
                                pos[qp][:],
                                wvs[d][:, o * P:(o + 1) * P],
                                ctxN[d][:, qp * 512:(qp + 1) * 512],
                                start=(d == 0), stop=(d == ND - 1))
                    for qp in range(2):
                        ob = out_pool.tile([P, 512], f32, name="ob", tag="ob")
                        nc.vector.tensor_mul(
                            ob[:], pos[qp][:],
                            bcsAll[:, qp * 512:(qp + 1) * 512])
                        nc.sync.dma_start(
                            outT_d[o * P:(o + 1) * P, qp * 512:(qp + 1) * 512],
                            ob[:])

    nc.compile()
    return nc


def _get_nc():
    if "nc" not in _NC_CACHE:
        _NC_CACHE["nc"] = _build_nc()
    return _NC_CACHE["nc"]


def _make_masks(h):
    """[128, 16*256] mask tile row: 1.0 where key 128c+p <= query 256g+j."""
    mk = np.zeros((P, 16 * 256), dtype=np.float16)
    p = np.arange(P)[:, None]
    j = np.arange(256)[None, :]
    for s in range(4):
        g = G[h][s]
        for m in range(4):
            c = L[s] - 4 + m
            mk[:, (4 * s + m) * 256:(4 * s + m + 1) * 256] = (
                (128 * c + p) <= (256 * g + j)).astype(np.float16)
    return mk


def kernel(x, W_q, W_k, W_v):
    from concourse.bass_utils import run_bass_kernel_spmd

    x = np.asarray(x, dtype=np.float32)
    x16 = x.astype(np.float16)
    wqT = np.ascontiguousarray(np.asarray(W_q, dtype=np.float32).T
                               .astype(np.float16))
    wkT = np.ascontiguousarray(np.asarray(W_k, dtype=np.float32).T
                               .astype(np.float16))
    wvT = np.ascontiguousarray(np.asarray(W_v, dtype=np.float32).T
                               .astype(np.float16))

    ones = np.zeros((P, 160), dtype=np.float16)
    ones[:, 0] = 1.0
    ones[0, 32:160] = 1.0
    masks_h = [_make_masks(0), _make_masks(1)]

    in_maps = []
    for b in range(B):
        xTb = np.ascontiguousarray(x16[b].T)
        for h in range(2):
            qcols = np.concatenate(
                [np.arange(g * 256, (g + 1) * 256) for g in G[h]])
            in_maps.append(dict(
                xqT=np.ascontiguousarray(xTb[:, qcols]),
                xT=xTb,
                xn=np.ascontiguousarray(x16[b]),
                wqT=wqT, wkT=wkT, wvT=wvT,
                masks=masks_h[h],
                ones=ones,
            ))

    nc = _get_nc()
    res = run_bass_kernel_spmd(nc, in_maps, core_ids=list(range(NCORES)),
                               trace=bool(os.environ.get("KERNEL_TRACE")))
    if os.environ.get("KERNEL_TRACE"):
        _NC_CACHE["last_results"] = res

    out = np.empty((B, S, DOUT), dtype=np.float32)
    for b in range(B):
        for h in range(2):
            oT = res.results[b * 2 + h]["outT"]
            for s2, g in enumerate(G[h]):
                out[b, g * 256:(g + 1) * 256, :] = \
                    oT[:, s2 * 256:(s2 + 1) * 256].T
    return out
